# revision 14
# baseline (speedup 1.0000x reference)
"""CapsuleLayer dynamic-routing kernel for 8 Trainium2 NeuronCores.

Problem: x[32, 2048, 16], W[1, 2048, 64, 32, 16] -> v[32, 64, 32]
  u_hat = einsum('iodk,bik->biod', W[0], x)
  3 routing iterations (softmax over out_caps, squash over out_dim).

Sharding: in_caps (i) split 8 ways (256/core); W shard SBUF-resident bf16.

v3 design notes: the DVE only engages its 2x bf16 packing mode on flat,
contiguous access patterns (v2 trace: flat multiply 858ns vs 3D/4D-view ops
stuck at 1x).  So every heavy DVE op here is flat:
  * column layout col = 64*d + o (d-major, o-minor over the whole tile):
    the d-reduction becomes flat half-tile adds (2x), finishing with a
    short strided reduce (f32 accumulate).
  * the softmax weight e/Z is DMA-broadcast to a full [128, 2048] bf16
    tile (stride-0 source AP) so the weighting multiply is flat 2x.
  * s/sacc layout [(j, b), 512] (j = d-octet): c_ij accumulation and the
    pass-1 dense contraction are 4x col-group-packed matmuls, 1 PSUM bank.
  * squash: per-partition partial norms + a tiny DMA regroup (d-octets
    live on different partition groups), small ops on [32, 64], then a
    DMA-replicated qq.
  * gpsimd owns the last 512 columns of the two big multiplies.

Routing state trick: b_ij(t) = sum_d u_hat * (v_0+...+v_{t-1}), so no
b_ij state is carried - only the accumulated V.
"""

import numpy as np
import ml_dtypes

B, IC, KD, OC, OD = 32, 2048, 16, 64, 32     # batch, in_caps, in_dim, out_caps, out_dim
NCORES = 8
ICC = IC // NCORES                            # 256 in_caps per core
NJ = ICC // 8                                 # 32 j-blocks (8 i per block)
OD2 = OC * OD                                 # 2048 flattened (o, d)
NUM_ROUTES = 3

_CACHE = {}


def _colmap():
    """newcol[o*OD + d] = 64*d + o  (d-major, o-minor)."""
    o = np.arange(OC)[:, None]
    d = np.arange(OD)[None, :]
    return (64 * d + o).reshape(-1)


def _build_program():
    import concourse.bacc as bacc
    import concourse.tile as tile
    import concourse.mybir as mybir

    f32 = mybir.dt.float32
    bf16 = mybir.dt.bfloat16
    ALU = mybir.AluOpType
    ACTF = mybir.ActivationFunctionType

    nc = bacc.Bacc("TRN2", target_bir_lowering=False, debug=False, num_devices=NCORES)

    WL_d = nc.dram_tensor("WL", [128, NJ * OD2], bf16, kind="ExternalInput").ap()
    xS0_d = nc.dram_tensor("xS0", [128, NJ * B], bf16, kind="ExternalInput").ap()
    xS1_d = nc.dram_tensor("xS1", [128, NJ * B], bf16, kind="ExternalInput").ap()
    SEL1_d = nc.dram_tensor("SEL1", [128, 32], bf16, kind="ExternalInput").ap()
    X2_d = nc.dram_tensor("X2", [128, NJ * B], bf16, kind="ExternalInput").ap()
    vout_d = nc.dram_tensor("v_out", [128, 512], f32, kind="ExternalOutput").ap()

    with tile.TileContext(nc) as tc:
        with (
            tc.tile_pool(name="const", bufs=1) as cp,
            tc.tile_pool(name="work", bufs=2) as wp,
            tc.tile_pool(name="small", bufs=2) as sp,
            tc.tile_pool(name="psum", bufs=7, space="PSUM") as pp,
            tc.tile_pool(name="psacc", bufs=1, space="PSUM") as pa,
            tc.tile_pool(name="dram", bufs=1, space="DRAM") as dp,
        ):
            # ---- resident inputs ----
            wl = cp.tile([128, NJ * OD2], bf16, tag="wl")
            for blk in range(8):
                w = NJ * OD2 // 8
                nc.sync.dma_start(out=wl[:, blk * w:(blk + 1) * w],
                                  in_=WL_d[:, blk * w:(blk + 1) * w])
            xs = [cp.tile([128, NJ * B], bf16, tag=f"xs{s}", name=f"xs{s}") for s in range(2)]
            nc.sync.dma_start(out=xs[0][:, :], in_=xS0_d[:, :])
            nc.sync.dma_start(out=xs[1][:, :], in_=xS1_d[:, :])
            sel1 = cp.tile([128, 32], bf16, tag="sel1")
            nc.sync.dma_start(out=sel1[:, :], in_=SEL1_d[:, :])
            x2t = cp.tile([128, NJ * B], bf16, tag="x2t")
            nc.sync.dma_start(out=x2t[:, :], in_=X2_d[:, :])

            # ---- persistent state ----
            V4 = cp.tile([128, OD2], bf16, tag="V4")     # V bf16, replicated x4
            Vacc = cp.tile([128, 512], f32, tag="Vacc")  # running sum of v_t [(j,b), 512]
            vb = cp.tile([128, 512], bf16, tag="vb")     # bf16 shadow of Vacc

            ar_in = [dp.tile([128, 512], f32, tag=f"ari{t}", name=f"ari{t}") for t in range(NUM_ROUTES)]
            ar_out = [dp.tile([128, 512], f32, tag=f"aro{t}", name=f"aro{t}") for t in range(NUM_ROUTES)]

            def allreduce_s(t, src_psum):
                """Evacuate s (psum [(j,b), 512]) -> allreduce -> s_sb."""
                s_sb = cp.tile([128, 512], f32, tag="ssb", name=f"s_sb{t}")
                nc.scalar.copy(s_sb[:, :], src_psum[:, :])
                nc.sync.dma_start(out=ar_in[t][:, :], in_=s_sb[:, :])
                nc.gpsimd.collective_compute(
                    "AllReduce", ALU.add,
                    replica_groups=[list(range(NCORES))],
                    ins=[ar_in[t].opt()],
                    outs=[ar_out[t].opt()],
                )
                nc.sync.dma_start(out=s_sb[:, :], in_=ar_out[t][:, :])
                return s_sb

            def squash(t, s_sb):
                """v_t = squash(s_sb); s_sb [(j,b), (d8,o64)]; j = d-octet.
                t<2: Vacc += v_t, V4 <- replicate(Vacc).  t==2: DMA to output."""
                sq = wp.tile([128, 512], f32, tag="sqv", name=f"sq{t}", bufs=1)
                nc.scalar.activation(sq[:, :], s_sb[:, :], ACTF.Square)
                # partial |s|^2 over this partition-group's 8 d's
                n2p = sp.tile([128, 64], f32, tag="n2p")
                nc.vector.tensor_reduce(
                    n2p[:, :], sq[:, :].rearrange("p (d o) -> p o d", o=64),
                    axis=mybir.AxisListType.X, op=ALU.add)
                # regroup the 4 d-octet partials onto batch partitions
                n2g = sp.tile([32, 256], f32, tag="n2g")
                for j in range(4):
                    nc.sync.dma_start(out=n2g[:, 64 * j:64 * (j + 1)],
                                      in_=n2p[32 * j:32 * j + 32, :])
                n2 = sp.tile([32, 64], f32, tag="n2")
                nc.vector.tensor_reduce(
                    n2[:, :], n2g[:, :].rearrange("p (j o) -> p o j", j=4),
                    axis=mybir.AxisListType.X, op=ALU.add)
                r0 = sp.tile([32, 64], f32, tag="r0")
                nc.scalar.activation(r0[:, :], n2[:, :], ACTF.Sqrt)
                # Newton polish: n = 0.5 * (r0 + n2 / r0)
                t1 = sp.tile([32, 64], f32, tag="t1")
                nc.vector.reciprocal(t1[:, :], r0[:, :])
                nc.vector.tensor_mul(t1[:, :], t1[:, :], n2[:, :])
                t2 = sp.tile([32, 64], f32, tag="t2")
                nc.vector.tensor_add(t2[:, :], t1[:, :], r0[:, :])
                nn = sp.tile([32, 64], f32, tag="nn")
                nc.vector.tensor_scalar_mul(nn[:, :], t2[:, :], 0.5)   # |s|
                den = sp.tile([32, 64], f32, tag="den")
                nc.vector.tensor_scalar_add(den[:, :], n2[:, :], 1.0)
                rec = sp.tile([32, 64], f32, tag="rec")
                nc.vector.reciprocal(rec[:, :], den[:, :])
                qq = sp.tile([32, 64], f32, tag="qq")
                nc.vector.tensor_mul(qq[:, :], nn[:, :], rec[:, :])  # |s|/(1+|s|^2)
                qq4 = sp.tile([128, 64], f32, tag="qq4")
                for j in range(4):
                    nc.sync.dma_start(out=qq4[32 * j:32 * j + 32, :], in_=qq[:, :])
                vt = wp.tile([128, 512], f32, tag="sqv", name=f"vt{t}", bufs=1)
                nc.vector.tensor_tensor(
                    out=vt[:, :].rearrange("p (d o) -> p d o", o=64),
                    in0=s_sb[:, :].rearrange("p (d o) -> p d o", o=64),
                    in1=qq4[:, :].unsqueeze(1).broadcast_to([128, 8, 64]),
                    op=ALU.mult)
                if t == NUM_ROUTES - 1:
                    nc.sync.dma_start(out=vout_d[:, :], in_=vt[:, :])
                else:
                    if t == 0:
                        nc.vector.tensor_copy(Vacc[:, :], vt[:, :])
                    else:
                        nc.vector.tensor_add(Vacc[:, :], Vacc[:, :], vt[:, :])
                    nc.vector.tensor_copy(vb[:, :], Vacc[:, :])
                    for g in range(4):
                        for j in range(4):
                            nc.sync.dma_start(
                                out=V4[32 * g:32 * g + 32, 512 * j:512 * (j + 1)],
                                in_=vb[32 * j:32 * j + 32, :])

            # ======== pass 1: s0 = sum_i u_hat / 64 ========
            sacc = pa.tile([128, 512], f32, tag="sacc", name="sacc0")
            for tau in range(NJ):
                for j in range(4):
                    nc.tensor.matmul(
                        sacc[32 * j:32 * j + 32, :],
                        lhsT=x2t[:, tau * B:(tau + 1) * B],
                        rhs=wl[:, tau * OD2 + j * 512: tau * OD2 + (j + 1) * 512],
                        start=(tau == 0), stop=(tau == NJ - 1),
                        tile_position=(0, 32 * j))
            s_sb = allreduce_s(0, sacc)
            squash(0, s_sb)

            # ======== passes 2..3: fused agreement/softmax/s ========
            # Software-pipelined by one quad: round q runs quad q's
            # matmuls/evac/agreement and quad q-1's softmax/weight/sel, so no
            # engine's strict FIFO head ever waits on the cross-engine chain.
            for t in range(1, NUM_ROUTES):
                sacc = pa.tile([128, 512], f32, tag="sacc", name=f"sacc{t}")
                NQ = 2 * NJ
                state = {}          # q -> (uhsb, agr)

                def stage_a(q):
                    """u_hat MMs + evac + agreement for quad q."""
                    jj, s_ = divmod(q, 2)
                    uh = [pp.tile([128, 512], f32, tag="uh", name=f"uh{t}_{q}_{c}")
                          for c in range(4)]
                    for c in range(4):
                        for r in range(4):
                            nc.tensor.matmul(
                                uh[c][32 * r:32 * r + 32, :],
                                lhsT=xs[s_][32 * r:32 * r + 32, jj * B:(jj + 1) * B],
                                rhs=wl[32 * r:32 * r + 32,
                                       jj * OD2 + c * 512: jj * OD2 + (c + 1) * 512],
                                start=True, stop=True,
                                tile_position=(32 * r, 32 * r),
                            )
                    uhsb = wp.tile([128, OD2], bf16, tag="uhb", name=f"uhsb{t}_{q}", bufs=4)
                    for c in range(4):
                        nc.scalar.copy(uhsb[:, c * 512:(c + 1) * 512], uh[c][:, :])
                    tmp = wp.tile([128, OD2], bf16, tag="tmp", name=f"tmp{t}_{q}")
                    nc.vector.tensor_mul(tmp[:, :1792], uhsb[:, :1792], V4[:, :1792])
                    nc.gpsimd.tensor_mul(tmp[:, 1792:], uhsb[:, 1792:], V4[:, 1792:])
                    tr1 = wp.tile([128, 1024], bf16, tag="tr1", name=f"tr1_{t}_{q}")
                    nc.vector.tensor_add(tr1[:, 0:768], tmp[:, 0:768], tmp[:, 1024:1792])
                    nc.gpsimd.tensor_add(tr1[:, 768:1024], tmp[:, 768:1024], tmp[:, 1792:2048])
                    tr2 = wp.tile([128, 512], bf16, tag="tr2", name=f"tr2_{t}_{q}")
                    nc.vector.tensor_add(tr2[:, 0:384], tr1[:, 0:384], tr1[:, 512:896])
                    nc.gpsimd.tensor_add(tr2[:, 384:512], tr1[:, 384:512], tr1[:, 896:1024])
                    agr = sp.tile([128, 64], f32, tag="agr", name=f"agr{t}_{q}", bufs=4)
                    nc.vector.tensor_reduce(
                        agr[:, :], tr2[:, :].rearrange("p (d o) -> p o d", o=64),
                        axis=mybir.AxisListType.X, op=ALU.add)
                    state[q] = (uhsb, agr)

                def stage_b(q):
                    """softmax + weight + s-accumulation for quad q."""
                    uhsb, agr = state.pop(q)
                    eB = sp.tile([128, 64], bf16, tag="eB")
                    Zs = sp.tile([128, 1], f32, tag="Zs")
                    nc.scalar.activation(eB[:, :], agr[:, :], ACTF.Exp,
                                         accum_out=Zs[:, :])
                    rZ = sp.tile([128, 1], f32, tag="rZ")
                    nc.vector.reciprocal(rZ[:, :], Zs[:, :])
                    eZ = sp.tile([128, 64], bf16, tag="eZ")
                    nc.vector.tensor_scalar_mul(eZ[:, :], eB[:, :], rZ[:, :])
                    tmp2 = wp.tile([128, OD2], bf16, tag="tmp2", name=f"tmp2_{t}_{q}")
                    nc.vector.tensor_tensor(
                        out=tmp2[:, :1792].rearrange("p (d o) -> p d o", o=64),
                        in0=uhsb[:, :1792].rearrange("p (d o) -> p d o", o=64),
                        in1=eZ[:, :].unsqueeze(1).broadcast_to([128, 28, 64]),
                        op=ALU.mult)
                    nc.gpsimd.tensor_tensor(
                        out=tmp2[:, 1792:].rearrange("p (d o) -> p d o", o=64),
                        in0=uhsb[:, 1792:].rearrange("p (d o) -> p d o", o=64),
                        in1=eZ[:, :].unsqueeze(1).broadcast_to([128, 4, 64]),
                        op=ALU.mult)
                    for j in range(4):
                        nc.tensor.matmul(
                            sacc[32 * j:32 * j + 32, :], lhsT=sel1[:, :],
                            rhs=tmp2[:, 512 * j:512 * (j + 1)],
                            start=(q == 0), stop=(q == NQ - 1),
                            tile_position=(0, 32 * j))

                for q in range(NQ):
                    stage_a(q)
                    if q > 2:
                        stage_b(q - 3)
                for qq_ in (NQ - 3, NQ - 2, NQ - 1):
                    stage_b(qq_)
                s_sb = allreduce_s(t, sacc)
                squash(t, s_sb)

    nc.compile()
    return nc


def _host_inputs(x, W):
    """Build per-core input maps (host-side relayout, not device time)."""
    W0 = np.asarray(W)[0]                       # [IC, OC, OD, KD]
    x = np.asarray(x)                           # [B, IC, KD]
    cmap = _colmap()                            # old od -> new col
    inv = np.empty_like(cmap)
    inv[cmap] = np.arange(OD2)                  # new col -> old od
    in_maps = []
    sel1 = np.zeros((128, 32), np.float32)
    for p in range(128):
        sel1[p, p % 32] = 1.0
    for c in range(NCORES):
        Wc = W0[c * ICC:(c + 1) * ICC].reshape(NJ, 8, OD2, KD)      # [tau, i8, od, k]
        Wc = Wc[:, :, inv, :]                                       # od axis -> new cols
        WL = np.ascontiguousarray(Wc.transpose(1, 3, 0, 2)          # [i8, k, tau, col]
                                  ).reshape(128, NJ * OD2)
        xc = x[:, c * ICC:(c + 1) * ICC, :].reshape(B, NJ, 8, KD)   # [b, tau, i8, k]
        xss = []
        for s in range(2):
            Xs = np.zeros((4, 2, KD, NJ, B), np.float32)            # [r, s', k, tau, b]
            Xs[:, s] = xc[:, :, s::2].transpose(2, 3, 1, 0)         # [r, k, tau, b]
            xss.append(Xs.reshape(128, NJ * B))
        X2 = (np.ascontiguousarray(xc.transpose(2, 3, 1, 0))        # [i8, k, tau, b]
              .reshape(128, NJ * B) / float(OC))
        in_maps.append({
            "WL": WL.astype(ml_dtypes.bfloat16),
            "xS0": xss[0].astype(ml_dtypes.bfloat16),
            "xS1": xss[1].astype(ml_dtypes.bfloat16),
            "SEL1": sel1.astype(ml_dtypes.bfloat16),
            "X2": X2.astype(ml_dtypes.bfloat16),
        })
    return in_maps


def kernel(x, W, _want_trace=False):
    from concourse.bass_utils import run_bass_kernel_spmd

    if "nc" not in _CACHE:
        _CACHE["nc"] = _build_program()
    nc = _CACHE["nc"]
    in_maps = _host_inputs(x, W)
    res = run_bass_kernel_spmd(nc, in_maps, core_ids=list(range(NCORES)),
                               trace=_want_trace)
    _CACHE["last_result"] = res
    out = np.asarray(res.results[0]["v_out"], np.float32)   # [(j,b), (d8, o64)]
    # out[32*j + b, 64*dlow + o] = v[b, o, 8*j + dlow]
    v = np.empty((B, OC, OD), np.float32)
    for j in range(4):
        blk = out[32 * j:32 * (j + 1)].reshape(B, 8, OC)    # [b, dlow, o]
        v[:, :, 8 * j:8 * (j + 1)] = blk.transpose(0, 2, 1)
    return v
